# revision 3
# baseline (speedup 1.0000x reference)
"""Trainium2 Bass kernel for nn_DualGCNModel (dual 2-layer GCN + MLP head).

Strategy (8 NeuronCores, single SPMD launch):
  - Nodes sharded by id: core c owns dst rows [c*12500, (c+1)*12500), padded
    to 12544 = 98 tiles of 128.
  - Edges sorted by dst on host; per dst-tile of 128 nodes, edges are split
    into K edge-tiles of 128. Normalization (D_in^-1/2, D_out^-1/2, degree
    clamp) is folded into one per-edge weight w_e = rs_in[dst]*rs_out[src],
    which multiplies the selection matrix used to aggregate.
  - Pass 1: gather concat(ori,struc)[src] rows (768B) via indirect DMA,
    aggregate with weighted selection matmuls into feature-major PSUM,
    apply layer-1 + layer-2 dense weights, write z = h1 @ W2 per node.
  - AllGather z across the 8 cores (on-chip collective).
  - Pass 2: gather z[src] rows (512B), aggregate with the same weights, add
    layer-2 biases, apply the MLP head, transpose, write output rows.

kernel(**inputs) takes the FULL un-sharded inputs and returns the full
(100000, 64) float32 output.
"""
import os
import sys

if "/opt/trn_rl_repo" not in sys.path:
    sys.path.insert(0, "/opt/trn_rl_repo")

from contextlib import ExitStack

import numpy as np

import concourse.bass as bass
import concourse.tile as tile
from concourse import bacc, mybir
from concourse.bass_utils import run_bass_kernel_spmd

P = 128
N_NODES = 100000
NCORES = 8
NPC = N_NODES // NCORES  # 12500 nodes per core
T = (NPC + P - 1) // P  # 98 dst tiles per core
PADN = T * P  # 12544 padded rows per core
D1 = 192  # pass-1 gather width (128 ori + 64 struc)
D2 = 128  # pass-2 gather width (64 + 64)
HID = 128
NCLS = 64
MLP_HID = 256
F32 = mybir.dt.float32
I32 = mybir.dt.int32

_BUILD_CACHE = {}
last_exec_ns = None


def _build(K):
    """Build the SPMD Bass module for K edge-tiles per dst tile."""
    nc = bacc.Bacc("TRN2", target_bir_lowering=False, debug=False, num_devices=NCORES)
    with tile.TileContext(nc) as tc, ExitStack() as ctx:
        # ---- I/O ----
        x_cat = nc.dram_tensor("x_cat", [N_NODES, D1], F32, kind="ExternalInput").ap()
        idx1 = nc.dram_tensor("idx1", [T, P, K], I32, kind="ExternalInput").ap()
        idx2 = nc.dram_tensor("idx2", [T, P, K], I32, kind="ExternalInput").ap()
        dloc = nc.dram_tensor("dloc", [T, P, K], F32, kind="ExternalInput").ap()
        warr = nc.dram_tensor("warr", [T, P, K], F32, kind="ExternalInput").ap()
        w1o = nc.dram_tensor("w1o", [HID, HID], F32, kind="ExternalInput").ap()
        w1s = nc.dram_tensor("w1s", [64, HID], F32, kind="ExternalInput").ap()
        w2o = nc.dram_tensor("w2o", [HID, NCLS], F32, kind="ExternalInput").ap()
        w2s = nc.dram_tensor("w2s", [HID, NCLS], F32, kind="ExternalInput").ap()
        wm1a = nc.dram_tensor("wm1a", [D2, P], F32, kind="ExternalInput").ap()
        wm1b = nc.dram_tensor("wm1b", [D2, P], F32, kind="ExternalInput").ap()
        wm2a = nc.dram_tensor("wm2a", [P, NCLS], F32, kind="ExternalInput").ap()
        wm2b = nc.dram_tensor("wm2b", [P, NCLS], F32, kind="ExternalInput").ap()
        b1o = nc.dram_tensor("b1o", [HID], F32, kind="ExternalInput").ap()
        b1s = nc.dram_tensor("b1s", [HID], F32, kind="ExternalInput").ap()
        b2c = nc.dram_tensor("b2c", [D2], F32, kind="ExternalInput").ap()
        bm1a = nc.dram_tensor("bm1a", [P], F32, kind="ExternalInput").ap()
        bm1b = nc.dram_tensor("bm1b", [P], F32, kind="ExternalInput").ap()
        bm2 = nc.dram_tensor("bm2", [NCLS], F32, kind="ExternalInput").ap()
        iota_in = nc.dram_tensor("iota", [P, P], F32, kind="ExternalInput").ap()
        ident_in = nc.dram_tensor("ident", [P, P], F32, kind="ExternalInput").ap()
        out_ext = nc.dram_tensor("out", [PADN, NCLS], F32, kind="ExternalOutput").ap()

        z_loc = nc.dram_tensor("z_loc", [PADN, D2], F32).ap()
        z_all = nc.dram_tensor("z_all", [NCORES * PADN, D2], F32, addr_space="Shared").ap()

        # ---- constant tiles ----
        wp = ctx.enter_context(tc.tile_pool(name="wp", bufs=1))
        w1o_sb = wp.tile([HID, HID], F32)
        nc.sync.dma_start(out=w1o_sb[:], in_=w1o[:])
        w1s_sb = wp.tile([64, HID], F32)
        nc.sync.dma_start(out=w1s_sb[:], in_=w1s[:])
        w2o_sb = wp.tile([HID, NCLS], F32)
        nc.sync.dma_start(out=w2o_sb[:], in_=w2o[:])
        w2s_sb = wp.tile([HID, NCLS], F32)
        nc.sync.dma_start(out=w2s_sb[:], in_=w2s[:])
        wm1a_sb = wp.tile([D2, P], F32)
        nc.sync.dma_start(out=wm1a_sb[:], in_=wm1a[:])
        wm1b_sb = wp.tile([D2, P], F32)
        nc.sync.dma_start(out=wm1b_sb[:], in_=wm1b[:])
        wm2a_sb = wp.tile([P, NCLS], F32)
        nc.sync.dma_start(out=wm2a_sb[:], in_=wm2a[:])
        wm2b_sb = wp.tile([P, NCLS], F32)
        nc.sync.dma_start(out=wm2b_sb[:], in_=wm2b[:])
        b1o_sb = wp.tile([HID, 1], F32)
        nc.sync.dma_start(out=b1o_sb[:], in_=b1o[:, None])
        b1s_sb = wp.tile([HID, 1], F32)
        nc.sync.dma_start(out=b1s_sb[:], in_=b1s[:, None])
        b2c_sb = wp.tile([D2, 1], F32)
        nc.sync.dma_start(out=b2c_sb[:], in_=b2c[:, None])
        bm1a_sb = wp.tile([P, 1], F32)
        nc.sync.dma_start(out=bm1a_sb[:], in_=bm1a[:, None])
        bm1b_sb = wp.tile([P, 1], F32)
        nc.sync.dma_start(out=bm1b_sb[:], in_=bm1b[:, None])
        bm2_sb = wp.tile([NCLS, 1], F32)
        nc.sync.dma_start(out=bm2_sb[:], in_=bm2[:, None])
        iota_sb = wp.tile([P, P], F32)
        nc.sync.dma_start(out=iota_sb[:], in_=iota_in[:])
        ident_sb = wp.tile([P, P], F32)
        nc.sync.dma_start(out=ident_sb[:], in_=ident_in[:])

        relu = mybir.ActivationFunctionType.Relu
        fcopy = mybir.ActivationFunctionType.Copy

        def build_sw(sp, metap, dloc_src, warr_src):
            """Load per-tile metadata and build the weighted selection matrix
            S_w[p, k*128+d] = w[p,k] * (dloc[p,k] == d)."""
            dl_t = metap.tile([P, K], F32, tag="dl")
            nc.sync.dma_start(out=dl_t[:], in_=dloc_src)
            w_t = metap.tile([P, K], F32, tag="w")
            nc.sync.dma_start(out=w_t[:], in_=warr_src)
            s01 = sp.tile([P, K * P], F32, tag="s01")
            nc.vector.tensor_tensor(
                out=s01[:].rearrange("p (k d) -> p k d", k=K),
                in0=dl_t[:].to_broadcast([P, K, P]),
                in1=iota_sb[:].rearrange("p (k d) -> p k d", k=1).to_broadcast([P, K, P]),
                op=mybir.AluOpType.is_equal,
            )
            sw = sp.tile([P, K * P], F32, tag="sw")
            nc.vector.tensor_tensor(
                out=sw[:].rearrange("p (k d) -> p k d", k=K),
                in0=s01[:].rearrange("p (k d) -> p k d", k=K),
                in1=w_t[:].to_broadcast([P, K, P]),
                op=mybir.AluOpType.mult,
            )
            return sw

        # ================= pass 1 =================
        with (
            tc.tile_pool(name="meta1", bufs=3) as metap,
            tc.tile_pool(name="sp1", bufs=2) as sp,
            tc.tile_pool(name="g1", bufs=8) as gp,
            tc.tile_pool(name="ip1", bufs=3) as ip,
            tc.tile_pool(name="wk1", bufs=3) as wk,
            tc.tile_pool(name="psA", bufs=2, space="PSUM") as psA,
            tc.tile_pool(name="psD1", bufs=2, space="PSUM") as psD,
        ):
            for t in range(T):
                idx_t = ip.tile([P, K], I32, tag="idx")
                nc.sync.dma_start(out=idx_t[:], in_=idx1[t])
                sw = build_sw(sp, metap, dloc[t], warr[t])
                seg_a = psA.tile([P, P], F32, tag="sega")
                seg_b = psA.tile([64, P], F32, tag="segb")
                for k in range(K):
                    g = gp.tile([P, D1], F32, tag="G")
                    nc.gpsimd.indirect_dma_start(
                        out=g[:],
                        out_offset=None,
                        in_=x_cat[:],
                        in_offset=bass.IndirectOffsetOnAxis(ap=idx_t[:, k : k + 1], axis=0),
                    )
                    nc.tensor.matmul(
                        out=seg_a[:],
                        lhsT=g[:, 0:HID],
                        rhs=sw[:, k * P : (k + 1) * P],
                        start=(k == 0),
                        stop=(k == K - 1),
                    )
                    nc.tensor.matmul(
                        out=seg_b[:],
                        lhsT=g[:, HID:D1],
                        rhs=sw[:, k * P : (k + 1) * P],
                        start=(k == 0),
                        stop=(k == K - 1),
                    )
                seg_a_sb = wk.tile([P, P], F32, tag="sega_sb")
                nc.vector.tensor_copy(out=seg_a_sb[:], in_=seg_a[:])
                seg_b_sb = wk.tile([64, P], F32, tag="segb_sb")
                nc.vector.tensor_copy(out=seg_b_sb[:], in_=seg_b[:])

                h1o_p = psD.tile([P, P], F32, tag="dp")
                nc.tensor.matmul(out=h1o_p[:], lhsT=w1o_sb[:], rhs=seg_a_sb[:], start=True, stop=True)
                h1o = wk.tile([P, P], F32, tag="h1o")
                nc.scalar.activation(h1o[:], h1o_p[:], relu, bias=b1o_sb[:])
                h1s_p = psD.tile([P, P], F32, tag="dp")
                nc.tensor.matmul(out=h1s_p[:], lhsT=w1s_sb[:], rhs=seg_b_sb[:], start=True, stop=True)
                h1s = wk.tile([P, P], F32, tag="h1s")
                nc.scalar.activation(h1s[:], h1s_p[:], relu, bias=b1s_sb[:])

                z_sb = wk.tile([P, P], F32, tag="z")
                zo_p = psD.tile([64, P], F32, tag="dp64")
                nc.tensor.matmul(out=zo_p[:], lhsT=w2o_sb[:], rhs=h1o[:], start=True, stop=True)
                nc.vector.tensor_copy(out=z_sb[0:64, :], in_=zo_p[:])
                zs_p = psD.tile([64, P], F32, tag="dp64")
                nc.tensor.matmul(out=zs_p[:], lhsT=w2s_sb[:], rhs=h1s[:], start=True, stop=True)
                nc.vector.tensor_copy(out=z_sb[64:128, :], in_=zs_p[:])

                zt_p = psD.tile([P, P], F32, tag="dp")
                nc.tensor.transpose(out=zt_p[:], in_=z_sb[:], identity=ident_sb[:])
                zt_sb = wk.tile([P, P], F32, tag="zt")
                nc.vector.tensor_copy(out=zt_sb[:], in_=zt_p[:])
                nc.sync.dma_start(out=z_loc[t * P : (t + 1) * P, :], in_=zt_sb[:])

        # ================= exchange =================
        nc.gpsimd.collective_compute(
            "AllGather",
            mybir.AluOpType.bypass,
            replica_groups=[list(range(NCORES))],
            ins=[z_loc[:].opt()],
            outs=[z_all[:].opt()],
        )

        # ================= pass 2 =================
        with (
            tc.tile_pool(name="meta2", bufs=3) as metap,
            tc.tile_pool(name="sp2", bufs=2) as sp,
            tc.tile_pool(name="g2", bufs=8) as gp,
            tc.tile_pool(name="ip2", bufs=3) as ip,
            tc.tile_pool(name="wk2", bufs=3) as wk,
            tc.tile_pool(name="psA2", bufs=2, space="PSUM") as psA,
            tc.tile_pool(name="psD2", bufs=2, space="PSUM") as psD,
        ):
            for t in range(T):
                idx_t = ip.tile([P, K], I32, tag="idx")
                nc.sync.dma_start(out=idx_t[:], in_=idx2[t])
                sw = build_sw(sp, metap, dloc[t], warr[t])
                seg2 = psA.tile([P, P], F32, tag="seg2")
                for k in range(K):
                    g = gp.tile([P, D2], F32, tag="G2")
                    nc.gpsimd.indirect_dma_start(
                        out=g[:],
                        out_offset=None,
                        in_=z_all[:],
                        in_offset=bass.IndirectOffsetOnAxis(ap=idx_t[:, k : k + 1], axis=0),
                    )
                    nc.tensor.matmul(
                        out=seg2[:],
                        lhsT=g[:],
                        rhs=sw[:, k * P : (k + 1) * P],
                        start=(k == 0),
                        stop=(k == K - 1),
                    )
                h2 = wk.tile([P, P], F32, tag="h2")
                nc.vector.tensor_tensor(
                    out=h2[:], in0=seg2[:], in1=b2c_sb[:].to_broadcast([P, P]),
                    op=mybir.AluOpType.add,
                )

                u0_p = psD.tile([P, P], F32, tag="dp")
                nc.tensor.matmul(out=u0_p[:], lhsT=wm1a_sb[:], rhs=h2[:], start=True, stop=True)
                u0 = wk.tile([P, P], F32, tag="u0")
                nc.scalar.activation(u0[:], u0_p[:], relu, bias=bm1a_sb[:])
                u1_p = psD.tile([P, P], F32, tag="dp")
                nc.tensor.matmul(out=u1_p[:], lhsT=wm1b_sb[:], rhs=h2[:], start=True, stop=True)
                u1 = wk.tile([P, P], F32, tag="u1")
                nc.scalar.activation(u1[:], u1_p[:], relu, bias=bm1b_sb[:])

                o_p = psD.tile([NCLS, P], F32, tag="dp64")
                nc.tensor.matmul(out=o_p[:], lhsT=wm2a_sb[:], rhs=u0[:], start=True, stop=False)
                nc.tensor.matmul(out=o_p[:], lhsT=wm2b_sb[:], rhs=u1[:], start=False, stop=True)
                o_t = wk.tile([NCLS, P], F32, tag="ot")
                nc.vector.tensor_tensor(
                    out=o_t[:], in0=o_p[:], in1=bm2_sb[:].to_broadcast([NCLS, P]),
                    op=mybir.AluOpType.add,
                )

                of_p = psD.tile([P, NCLS], F32, tag="dpT")
                nc.tensor.transpose(out=of_p[:], in_=o_t[:], identity=ident_sb[:NCLS, :NCLS])
                o_sb = wk.tile([P, NCLS], F32, tag="osb")
                nc.vector.tensor_copy(out=o_sb[:], in_=of_p[:])
                nc.sync.dma_start(out=out_ext[t * P : (t + 1) * P, :], in_=o_sb[:])

    nc.compile()
    return nc


def _host_prep(src, dst, ori_feat, struc_feat):
    src = np.asarray(src).astype(np.int64)
    dst = np.asarray(dst).astype(np.int64)
    n = N_NODES
    deg_out = np.bincount(src, minlength=n).astype(np.float64)
    deg_in = np.bincount(dst, minlength=n).astype(np.float64)
    rs_out = (1.0 / np.sqrt(np.clip(deg_out, 1.0, None))).astype(np.float32)
    rs_in = (1.0 / np.sqrt(np.clip(deg_in, 1.0, None))).astype(np.float32)
    w_all = rs_in[dst] * rs_out[src]

    order = np.argsort(dst, kind="stable")
    src_s = src[order]
    dst_s = dst[order]
    w_s = w_all[order]

    core = dst_s // NPC
    local = dst_s - core * NPC
    tile_id = local // P
    dst_local = (local % P).astype(np.float32)
    group = core * T + tile_id  # global (core, tile) group, sorted ascending

    counts = np.bincount(group, minlength=NCORES * T)
    K = int(max(1, int(np.ceil(counts.max() / P))))

    starts = np.zeros(NCORES * T + 1, np.int64)
    np.cumsum(counts, out=starts[1:])
    j_within = np.arange(len(src_s)) - starts[group]
    kk = j_within // P
    pp = j_within % P

    idx1 = np.zeros((NCORES, T, P, K), np.int32)
    idx2 = np.zeros((NCORES, T, P, K), np.int32)
    dl = np.full((NCORES, T, P, K), 200.0, np.float32)
    wa = np.zeros((NCORES, T, P, K), np.float32)
    c = core.astype(np.int64)
    t_ = tile_id.astype(np.int64)
    idx1[c, t_, pp, kk] = src_s.astype(np.int32)
    idx2[c, t_, pp, kk] = ((src_s // NPC) * PADN + (src_s % NPC)).astype(np.int32)
    dl[c, t_, pp, kk] = dst_local
    wa[c, t_, pp, kk] = w_s

    x_cat = np.concatenate(
        [np.asarray(ori_feat, np.float32), np.asarray(struc_feat, np.float32)], axis=1
    )
    x_cat = np.ascontiguousarray(x_cat, np.float32)
    return K, x_cat, idx1, idx2, dl, wa


def kernel(src, dst, ori_feat, struc_feat,
           W1o, b1o, W2o, b2o, W1s, b1s, W2s, b2s,
           Wm1, bm1, Wm2, bm2):
    global last_exec_ns
    K, x_cat, idx1, idx2, dl, wa = _host_prep(src, dst, ori_feat, struc_feat)

    if K not in _BUILD_CACHE:
        _BUILD_CACHE[K] = _build(K)
    nc = _BUILD_CACHE[K]

    f = lambda a: np.ascontiguousarray(np.asarray(a), dtype=np.float32)
    Wm1 = f(Wm1)
    Wm2 = f(Wm2)
    shared = {
        "x_cat": x_cat,
        "w1o": f(W1o), "w1s": f(W1s), "w2o": f(W2o), "w2s": f(W2s),
        "wm1a": f(Wm1[:, :P]), "wm1b": f(Wm1[:, P:]),
        "wm2a": f(Wm2[:P, :]), "wm2b": f(Wm2[P:, :]),
        "b1o": f(b1o), "b1s": f(b1s),
        "b2c": np.concatenate([f(b2o), f(b2s)]),
        "bm1a": f(bm1)[:P], "bm1b": f(bm1)[P:],
        "bm2": f(bm2),
        "iota": np.broadcast_to(np.arange(P, dtype=np.float32), (P, P)).copy(),
        "ident": np.eye(P, dtype=np.float32),
    }
    in_maps = [
        {**shared, "idx1": idx1[c], "idx2": idx2[c], "dloc": dl[c], "warr": wa[c]}
        for c in range(NCORES)
    ]
    trace = bool(os.environ.get("BASS_TRACE"))
    r = run_bass_kernel_spmd(nc, in_maps, list(range(NCORES)), trace=trace)
    last_exec_ns = r.exec_time_ns

    out = np.empty((N_NODES, NCLS), np.float32)
    for c in range(NCORES):
        out[c * NPC : (c + 1) * NPC] = np.asarray(r.results[c]["out"]).reshape(PADN, NCLS)[:NPC]
    return out


# revision 5
# speedup vs baseline: 1.0490x; 1.0490x over previous
"""Trainium2 Bass kernel for nn_DualGCNModel (dual 2-layer GCN + MLP head).

Strategy (8 NeuronCores, single SPMD launch):
  - Nodes sharded by id: core c owns dst rows [c*12500, (c+1)*12500), padded
    to 12544 = 98 tiles of 128.
  - Edges sorted by dst on host; per dst-tile of 128 nodes, edges are split
    into K edge-tiles of 128. Normalization (D_in^-1/2, D_out^-1/2, degree
    clamp) is folded into one per-edge weight w_e = rs_in[dst]*rs_out[src],
    which multiplies the selection matrix used to aggregate.
  - Pass 1: gather concat(ori,struc)[src] rows (768B) via indirect DMA,
    aggregate with weighted selection matmuls into feature-major PSUM,
    apply layer-1 + layer-2 dense weights, write z = h1 @ W2 per node.
  - AllGather z across the 8 cores (on-chip collective).
  - Pass 2: gather z[src] rows (512B), aggregate with the same weights, add
    layer-2 biases, apply the MLP head, transpose, write output rows.

kernel(**inputs) takes the FULL un-sharded inputs and returns the full
(100000, 64) float32 output.
"""
import os
import sys

if "/opt/trn_rl_repo" not in sys.path:
    sys.path.insert(0, "/opt/trn_rl_repo")

from contextlib import ExitStack

import numpy as np

import concourse.bass as bass
import concourse.tile as tile
from concourse import bacc, mybir
from concourse.bass_utils import run_bass_kernel_spmd

P = 128
N_NODES = 100000
NCORES = 8
NPC = N_NODES // NCORES  # 12500 nodes per core
T = (NPC + P - 1) // P  # 98 dst tiles per core
PADN = T * P  # 12544 padded rows per core
D1 = 192  # pass-1 gather width (128 ori + 64 struc)
D2 = 128  # pass-2 gather width (64 + 64)
HID = 128
NCLS = 64
MLP_HID = 256
F32 = mybir.dt.float32
I32 = mybir.dt.int32

_BUILD_CACHE = {}
last_exec_ns = None


def _build(Ks):
    """Build the SPMD Bass module; Ks[t] = edge-tiles for dst tile t."""
    KMAX = max(Ks)
    nc = bacc.Bacc("TRN2", target_bir_lowering=False, debug=False, num_devices=NCORES)
    with tile.TileContext(nc) as tc, ExitStack() as ctx:
        # ---- I/O ----
        x_cat = nc.dram_tensor("x_cat", [N_NODES, D1], F32, kind="ExternalInput").ap()
        idx1 = nc.dram_tensor("idx1", [T, P, KMAX], I32, kind="ExternalInput").ap()
        idx2 = nc.dram_tensor("idx2", [T, P, KMAX], I32, kind="ExternalInput").ap()
        dloc = nc.dram_tensor("dloc", [T, P, KMAX], F32, kind="ExternalInput").ap()
        warr = nc.dram_tensor("warr", [T, P, KMAX], F32, kind="ExternalInput").ap()
        w1o = nc.dram_tensor("w1o", [HID, HID], F32, kind="ExternalInput").ap()
        w1s = nc.dram_tensor("w1s", [64, HID], F32, kind="ExternalInput").ap()
        w2o = nc.dram_tensor("w2o", [HID, NCLS], F32, kind="ExternalInput").ap()
        w2s = nc.dram_tensor("w2s", [HID, NCLS], F32, kind="ExternalInput").ap()
        wm1a = nc.dram_tensor("wm1a", [D2, P], F32, kind="ExternalInput").ap()
        wm1b = nc.dram_tensor("wm1b", [D2, P], F32, kind="ExternalInput").ap()
        wm2a = nc.dram_tensor("wm2a", [P, NCLS], F32, kind="ExternalInput").ap()
        wm2b = nc.dram_tensor("wm2b", [P, NCLS], F32, kind="ExternalInput").ap()
        b1o = nc.dram_tensor("b1o", [HID], F32, kind="ExternalInput").ap()
        b1s = nc.dram_tensor("b1s", [HID], F32, kind="ExternalInput").ap()
        b2c = nc.dram_tensor("b2c", [D2], F32, kind="ExternalInput").ap()
        bm1a = nc.dram_tensor("bm1a", [P], F32, kind="ExternalInput").ap()
        bm1b = nc.dram_tensor("bm1b", [P], F32, kind="ExternalInput").ap()
        bm2 = nc.dram_tensor("bm2", [NCLS], F32, kind="ExternalInput").ap()
        iota_in = nc.dram_tensor("iota", [P, P], F32, kind="ExternalInput").ap()
        ident_in = nc.dram_tensor("ident", [P, P], F32, kind="ExternalInput").ap()
        out_ext = nc.dram_tensor("out", [PADN, NCLS], F32, kind="ExternalOutput").ap()

        z_loc = nc.dram_tensor("z_loc", [PADN, D2], F32).ap()
        z_all = nc.dram_tensor("z_all", [NCORES * PADN, D2], F32).ap()

        # ---- constant tiles ----
        wp = ctx.enter_context(tc.tile_pool(name="wp", bufs=1))
        w1o_sb = wp.tile([HID, HID], F32)
        nc.sync.dma_start(out=w1o_sb[:], in_=w1o[:])
        w1s_sb = wp.tile([64, HID], F32)
        nc.sync.dma_start(out=w1s_sb[:], in_=w1s[:])
        w2o_sb = wp.tile([HID, NCLS], F32)
        nc.sync.dma_start(out=w2o_sb[:], in_=w2o[:])
        w2s_sb = wp.tile([HID, NCLS], F32)
        nc.sync.dma_start(out=w2s_sb[:], in_=w2s[:])
        wm1a_sb = wp.tile([D2, P], F32)
        nc.sync.dma_start(out=wm1a_sb[:], in_=wm1a[:])
        wm1b_sb = wp.tile([D2, P], F32)
        nc.sync.dma_start(out=wm1b_sb[:], in_=wm1b[:])
        wm2a_sb = wp.tile([P, NCLS], F32)
        nc.sync.dma_start(out=wm2a_sb[:], in_=wm2a[:])
        wm2b_sb = wp.tile([P, NCLS], F32)
        nc.sync.dma_start(out=wm2b_sb[:], in_=wm2b[:])
        b1o_sb = wp.tile([HID, 1], F32)
        nc.sync.dma_start(out=b1o_sb[:], in_=b1o[:, None])
        b1s_sb = wp.tile([HID, 1], F32)
        nc.sync.dma_start(out=b1s_sb[:], in_=b1s[:, None])
        b2c_sb = wp.tile([D2, 1], F32)
        nc.sync.dma_start(out=b2c_sb[:], in_=b2c[:, None])
        bm1a_sb = wp.tile([P, 1], F32)
        nc.sync.dma_start(out=bm1a_sb[:], in_=bm1a[:, None])
        bm1b_sb = wp.tile([P, 1], F32)
        nc.sync.dma_start(out=bm1b_sb[:], in_=bm1b[:, None])
        bm2_sb = wp.tile([NCLS, 1], F32)
        nc.sync.dma_start(out=bm2_sb[:], in_=bm2[:, None])
        iota_sb = wp.tile([P, P], F32)
        nc.sync.dma_start(out=iota_sb[:], in_=iota_in[:])
        ident_sb = wp.tile([P, P], F32)
        nc.sync.dma_start(out=ident_sb[:], in_=ident_in[:])

        relu = mybir.ActivationFunctionType.Relu
        fcopy = mybir.ActivationFunctionType.Copy

        def build_sw(sp, metap, dloc_src, warr_src, K):
            """Load per-tile metadata and build the weighted selection matrix
            S_w[p, k*128+d] = w[p,k] * (dloc[p,k] == d)."""
            dl_t = metap.tile([P, KMAX], F32, tag="dl", name="dl_t")[:, :K]
            nc.scalar.dma_start(out=dl_t[:], in_=dloc_src)
            w_t = metap.tile([P, KMAX], F32, tag="w", name="w_t")[:, :K]
            nc.scalar.dma_start(out=w_t[:], in_=warr_src)
            s01 = sp.tile([P, KMAX * P], F32, tag="s01", name="s01")[:, : K * P]
            nc.vector.tensor_tensor(
                out=s01[:].rearrange("p (k d) -> p k d", k=K),
                in0=dl_t[:].to_broadcast([P, K, P]),
                in1=iota_sb[:].rearrange("p (k d) -> p k d", k=1).to_broadcast([P, K, P]),
                op=mybir.AluOpType.is_equal,
            )
            sw = sp.tile([P, KMAX * P], F32, tag="sw", name="sw")[:, : K * P]
            nc.vector.tensor_tensor(
                out=sw[:].rearrange("p (k d) -> p k d", k=K),
                in0=s01[:].rearrange("p (k d) -> p k d", k=K),
                in1=w_t[:].to_broadcast([P, K, P]),
                op=mybir.AluOpType.mult,
            )
            return sw

        # ================= pass 1 =================
        with (
            tc.tile_pool(name="meta1", bufs=6) as metap,
            tc.tile_pool(name="sp1", bufs=3) as sp,
            tc.tile_pool(name="g1", bufs=12) as gp,
            tc.tile_pool(name="ip1", bufs=6) as ip,
            tc.tile_pool(name="wk1", bufs=3) as wk,
            tc.tile_pool(name="psA", bufs=2, space="PSUM") as psA,
            tc.tile_pool(name="psD1", bufs=2, space="PSUM") as psD,
        ):
            for t in range(T):
                K = Ks[t]
                idx_t = ip.tile([P, KMAX], I32, tag="idx", name="idx_t")[:, :K]
                nc.scalar.dma_start(out=idx_t[:], in_=idx1[t][:, :K])
                sw = build_sw(sp, metap, dloc[t][:, :K], warr[t][:, :K], K)
                seg_a = psA.tile([P, P], F32, tag="sega")
                seg_b = psA.tile([64, P], F32, tag="segb")
                for k in range(K):
                    g = gp.tile([P, D1], F32, tag="G")
                    nc.gpsimd.indirect_dma_start(
                        out=g[:],
                        out_offset=None,
                        in_=x_cat[:],
                        in_offset=bass.IndirectOffsetOnAxis(ap=idx_t[:, k : k + 1], axis=0),
                    )
                    nc.tensor.matmul(
                        out=seg_a[:],
                        lhsT=g[:, 0:HID],
                        rhs=sw[:, k * P : (k + 1) * P],
                        start=(k == 0),
                        stop=(k == K - 1),
                    )
                    nc.tensor.matmul(
                        out=seg_b[:],
                        lhsT=g[:, HID:D1],
                        rhs=sw[:, k * P : (k + 1) * P],
                        start=(k == 0),
                        stop=(k == K - 1),
                    )
                seg_a_sb = wk.tile([P, P], F32, tag="sega_sb")
                nc.vector.tensor_copy(out=seg_a_sb[:], in_=seg_a[:])
                seg_b_sb = wk.tile([64, P], F32, tag="segb_sb")
                nc.vector.tensor_copy(out=seg_b_sb[:], in_=seg_b[:])

                h1o_p = psD.tile([P, P], F32, tag="dp")
                nc.tensor.matmul(out=h1o_p[:], lhsT=w1o_sb[:], rhs=seg_a_sb[:], start=True, stop=True)
                h1o = wk.tile([P, P], F32, tag="h1o")
                nc.scalar.activation(h1o[:], h1o_p[:], relu, bias=b1o_sb[:])
                h1s_p = psD.tile([P, P], F32, tag="dp")
                nc.tensor.matmul(out=h1s_p[:], lhsT=w1s_sb[:], rhs=seg_b_sb[:], start=True, stop=True)
                h1s = wk.tile([P, P], F32, tag="h1s")
                nc.scalar.activation(h1s[:], h1s_p[:], relu, bias=b1s_sb[:])

                z_sb = wk.tile([P, P], F32, tag="z")
                zo_p = psD.tile([64, P], F32, tag="dp64")
                nc.tensor.matmul(out=zo_p[:], lhsT=w2o_sb[:], rhs=h1o[:], start=True, stop=True)
                nc.vector.tensor_copy(out=z_sb[0:64, :], in_=zo_p[:])
                zs_p = psD.tile([64, P], F32, tag="dp64")
                nc.tensor.matmul(out=zs_p[:], lhsT=w2s_sb[:], rhs=h1s[:], start=True, stop=True)
                nc.vector.tensor_copy(out=z_sb[64:128, :], in_=zs_p[:])

                zt_p = psD.tile([P, P], F32, tag="dp")
                nc.tensor.transpose(out=zt_p[:], in_=z_sb[:], identity=ident_sb[:])
                zt_sb = wk.tile([P, P], F32, tag="zt")
                nc.vector.tensor_copy(out=zt_sb[:], in_=zt_p[:])
                nc.sync.dma_start(out=z_loc[t * P : (t + 1) * P, :], in_=zt_sb[:])

        # ================= exchange =================
        nc.gpsimd.collective_compute(
            "AllGather",
            mybir.AluOpType.bypass,
            replica_groups=[list(range(NCORES))],
            ins=[z_loc[:].opt()],
            outs=[z_all[:].opt()],
        )

        # ================= pass 2 =================
        with (
            tc.tile_pool(name="meta2", bufs=6) as metap,
            tc.tile_pool(name="sp2", bufs=3) as sp,
            tc.tile_pool(name="g2", bufs=12) as gp,
            tc.tile_pool(name="ip2", bufs=6) as ip,
            tc.tile_pool(name="wk2", bufs=3) as wk,
            tc.tile_pool(name="psA2", bufs=2, space="PSUM") as psA,
            tc.tile_pool(name="psD2", bufs=2, space="PSUM") as psD,
        ):
            for t in range(T):
                K = Ks[t]
                idx_t = ip.tile([P, KMAX], I32, tag="idx", name="idx_t")[:, :K]
                nc.scalar.dma_start(out=idx_t[:], in_=idx2[t][:, :K])
                sw = build_sw(sp, metap, dloc[t][:, :K], warr[t][:, :K], K)
                seg2 = psA.tile([P, P], F32, tag="seg2")
                for k in range(K):
                    g = gp.tile([P, D2], F32, tag="G2")
                    nc.gpsimd.indirect_dma_start(
                        out=g[:],
                        out_offset=None,
                        in_=z_all[:],
                        in_offset=bass.IndirectOffsetOnAxis(ap=idx_t[:, k : k + 1], axis=0),
                    )
                    nc.tensor.matmul(
                        out=seg2[:],
                        lhsT=g[:],
                        rhs=sw[:, k * P : (k + 1) * P],
                        start=(k == 0),
                        stop=(k == K - 1),
                    )
                h2 = wk.tile([P, P], F32, tag="h2")
                nc.vector.tensor_tensor(
                    out=h2[:], in0=seg2[:], in1=b2c_sb[:].to_broadcast([P, P]),
                    op=mybir.AluOpType.add,
                )

                u0_p = psD.tile([P, P], F32, tag="dp")
                nc.tensor.matmul(out=u0_p[:], lhsT=wm1a_sb[:], rhs=h2[:], start=True, stop=True)
                u0 = wk.tile([P, P], F32, tag="u0")
                nc.scalar.activation(u0[:], u0_p[:], relu, bias=bm1a_sb[:])
                u1_p = psD.tile([P, P], F32, tag="dp")
                nc.tensor.matmul(out=u1_p[:], lhsT=wm1b_sb[:], rhs=h2[:], start=True, stop=True)
                u1 = wk.tile([P, P], F32, tag="u1")
                nc.scalar.activation(u1[:], u1_p[:], relu, bias=bm1b_sb[:])

                o_p = psD.tile([NCLS, P], F32, tag="dp64")
                nc.tensor.matmul(out=o_p[:], lhsT=wm2a_sb[:], rhs=u0[:], start=True, stop=False)
                nc.tensor.matmul(out=o_p[:], lhsT=wm2b_sb[:], rhs=u1[:], start=False, stop=True)
                o_t = wk.tile([NCLS, P], F32, tag="ot")
                nc.vector.tensor_tensor(
                    out=o_t[:], in0=o_p[:], in1=bm2_sb[:].to_broadcast([NCLS, P]),
                    op=mybir.AluOpType.add,
                )

                of_p = psD.tile([P, NCLS], F32, tag="dpT")
                nc.tensor.transpose(out=of_p[:], in_=o_t[:], identity=ident_sb[:NCLS, :NCLS])
                o_sb = wk.tile([P, NCLS], F32, tag="osb")
                nc.vector.tensor_copy(out=o_sb[:], in_=of_p[:])
                nc.sync.dma_start(out=out_ext[t * P : (t + 1) * P, :], in_=o_sb[:])

    nc.compile()
    return nc


def _host_prep(src, dst, ori_feat, struc_feat):
    src = np.asarray(src).astype(np.int64)
    dst = np.asarray(dst).astype(np.int64)
    n = N_NODES
    deg_out = np.bincount(src, minlength=n).astype(np.float64)
    deg_in = np.bincount(dst, minlength=n).astype(np.float64)
    rs_out = (1.0 / np.sqrt(np.clip(deg_out, 1.0, None))).astype(np.float32)
    rs_in = (1.0 / np.sqrt(np.clip(deg_in, 1.0, None))).astype(np.float32)
    w_all = rs_in[dst] * rs_out[src]

    order = np.argsort(dst, kind="stable")
    src_s = src[order]
    dst_s = dst[order]
    w_s = w_all[order]

    core = dst_s // NPC
    local = dst_s - core * NPC
    tile_id = local // P
    dst_local = (local % P).astype(np.float32)
    group = core * T + tile_id  # global (core, tile) group, sorted ascending

    counts = np.bincount(group, minlength=NCORES * T)
    per_tile = counts.reshape(NCORES, T).max(axis=0)
    Ks = tuple(int(max(1, np.ceil(c / P))) for c in per_tile)
    K = int(max(Ks))

    starts = np.zeros(NCORES * T + 1, np.int64)
    np.cumsum(counts, out=starts[1:])
    j_within = np.arange(len(src_s)) - starts[group]
    kk = j_within // P
    pp = j_within % P

    idx1 = np.zeros((NCORES, T, P, K), np.int32)
    idx2 = np.zeros((NCORES, T, P, K), np.int32)
    dl = np.full((NCORES, T, P, K), 200.0, np.float32)
    wa = np.zeros((NCORES, T, P, K), np.float32)
    c = core.astype(np.int64)
    t_ = tile_id.astype(np.int64)
    idx1[c, t_, pp, kk] = src_s.astype(np.int32)
    idx2[c, t_, pp, kk] = ((src_s // NPC) * PADN + (src_s % NPC)).astype(np.int32)
    dl[c, t_, pp, kk] = dst_local
    wa[c, t_, pp, kk] = w_s

    x_cat = np.concatenate(
        [np.asarray(ori_feat, np.float32), np.asarray(struc_feat, np.float32)], axis=1
    )
    x_cat = np.ascontiguousarray(x_cat, np.float32)
    return Ks, K, x_cat, idx1, idx2, dl, wa


def kernel(src, dst, ori_feat, struc_feat,
           W1o, b1o, W2o, b2o, W1s, b1s, W2s, b2s,
           Wm1, bm1, Wm2, bm2):
    global last_exec_ns
    Ks, K, x_cat, idx1, idx2, dl, wa = _host_prep(src, dst, ori_feat, struc_feat)

    if Ks not in _BUILD_CACHE:
        _BUILD_CACHE[Ks] = _build(Ks)
    nc = _BUILD_CACHE[Ks]

    f = lambda a: np.ascontiguousarray(np.asarray(a), dtype=np.float32)
    Wm1 = f(Wm1)
    Wm2 = f(Wm2)
    shared = {
        "x_cat": x_cat,
        "w1o": f(W1o), "w1s": f(W1s), "w2o": f(W2o), "w2s": f(W2s),
        "wm1a": f(Wm1[:, :P]), "wm1b": f(Wm1[:, P:]),
        "wm2a": f(Wm2[:P, :]), "wm2b": f(Wm2[P:, :]),
        "b1o": f(b1o), "b1s": f(b1s),
        "b2c": np.concatenate([f(b2o), f(b2s)]),
        "bm1a": f(bm1)[:P], "bm1b": f(bm1)[P:],
        "bm2": f(bm2),
        "iota": np.broadcast_to(np.arange(P, dtype=np.float32), (P, P)).copy(),
        "ident": np.eye(P, dtype=np.float32),
    }
    in_maps = [
        {**shared, "idx1": idx1[c], "idx2": idx2[c], "dloc": dl[c], "warr": wa[c]}
        for c in range(NCORES)
    ]
    trace = bool(os.environ.get("BASS_TRACE"))
    r = run_bass_kernel_spmd(nc, in_maps, list(range(NCORES)), trace=trace)
    last_exec_ns = r.exec_time_ns

    out = np.empty((N_NODES, NCLS), np.float32)
    for c in range(NCORES):
        out[c * NPC : (c + 1) * NPC] = np.asarray(r.results[c]["out"]).reshape(PADN, NCLS)[:NPC]
    return out
